# revision 30
# baseline (speedup 1.0000x reference)
"""ContrastiveDist kernel for TRN2 (8 NeuronCores, SPMD) -- v4.9.

Measured: 38732 ns HW exec (core 0), rel err 3.23e-3 (vs 44673 ns
baseline v3).

out[n] = sum_e -(t_e . v_n) / (||t_e|| * ||v_n|| + eps)
       = (s . v_n) / ||v_n||      with s = -sum_e t_e / ||t_e||
(eps shifts the result by ~4e-11 relative -- dropped.)

Schedule design (from the v3 / v4.0-v4.4 traces):
 * THREE DMA queues: SP HWDGE (nc.sync), ACT HWDGE (nc.scalar), GPSIMD
   SWDGE (nc.gpsimd); ~285 GB/s aggregate HBM-bound, round-robin per
   packet.  tgt quarters head both HWDGE rings.  ALL DMA issues carry
   high_priority: they are data-ready at sim t=0, so the static
   scheduler always places them ahead of (mispredicted) compute --
   v4.4's XD issue sat behind phase-A ACT work until 19.6us.
 * eye and dotw are built ON DEVICE with strided memsets/copies
   (diagonal of a [128, 49] tile = stride-8 free-axis slice) -- no eye
   DMA, no eye dependency in the s chain, and the GP ring's first chunk
   is a clean single node block.
 * target entity-major [128e, 16, 256d], 4 quarters: DVE square+reduce
   for q0/q1, GPSIMD square -> DVE reduce for q2/q3 (ACT Square with
   accum_out works but costs 769ns/tile and jams the ACT queue ahead
   of DMA issues -- removed in v4.7).  ACT Abs_reciprocal_sqrt emits
   winv in BF16 directly; the s sign folds into ACT Copy(scale=-1)
   column copies.
 * GP ring: first block immediately, bulk chunks issued after the
   gpsimd phase-A square so tgt keeps the early bandwidth.
 * PE prewarm matmuls run while the first DMAs stream (HAM clock gate
   wants ~3.4us of activity).  NOTE: W-hinted "filler" matmuls placed
   mid-kernel got mis-scheduled into the node matmul stream and COST
   ~1-2us (v4.7) -- do not re-add.
 * phase-A s accumulation uses the v3 column scheme: 32 matmuls with
   [128e, 128d] weight loads and 1-column rhs pipeline at ~27ns each
   and write s_col psum directly (no row->column conversion chain).
 * psum pairs: blocks 0-6 / 7-13; block-diag lhsT routes block b to its
   psum row; tails are ACT arsqrt [7,448] + one DVE mul; outs on SP.
 * fused DVE tensor_tensor_reduce is NOT used anywhere (locks up HW).
"""

import numpy as np
import ml_dtypes
from contextlib import ExitStack

import concourse.bacc as bacc
import concourse.bass as bass
import concourse.mybir as mybir
import concourse.tile as tile
from concourse import bass_utils

E, D = 2048, 256
N_FULL = 50000
N_CORES = 8
NPC = N_FULL // N_CORES
G = 448
NG = 14
NPAD = G * NG
NP = 7
A = 2
ET = E // 128
EYC = NP * NP            # eye tile columns (on-device)
TQ = 4
H = ET // TQ
WARM_MM = 6

# node chunks: (ring, [blocks], square engine 'V'/'S', est arrival us,
#               issue early?)
CHUNKS = [
    ("G", [0], "S", 10.7, True),            # GE
    ("G", [1, 2], "S", 14.2, True),         # GA
    ("G", [3, 4], "V", 20.0, False),        # GB (issued after gp TT q3)
    ("G", [5], "V", 22.2, False),           # GC
    ("S", [6, 7, 8], "V", 20.0, True),      # SA
    ("A", [9, 10, 11], "S", 20.0, True),    # XA
    ("S", [12], "S", 23.0, True),           # SD
    ("A", [13], "V", 23.0, True),           # XD
]
DOT_ORDER = [0, 1, 2, 3, 4, 6, 7, 8, 9, 10, 11, 5, 12, 13]
SSQ_ORDER = [0, 1, 2, 3, 4, 6, 7, 8, 9, 10, 11, 5, 12, 13]
ARRIVE = {}
for _ring, _bs, _sq, _t, _e in CHUNKS:
    for _b in _bs:
        ARRIVE[_b] = _t
S_READY = 18.4

F32 = mybir.dt.float32
BF16 = mybir.dt.bfloat16
BF = ml_dtypes.bfloat16
ARSQRT = mybir.ActivationFunctionType.Abs_reciprocal_sqrt
SQUARE = mybir.ActivationFunctionType.Square
COPY = mybir.ActivationFunctionType.Copy

_cache = {}


def _build():
    nc = bacc.Bacc(
        "TRN2",
        target_bir_lowering=False,
        debug=False,
        enable_asserts=True,
        num_devices=N_CORES,
    )
    tgt = nc.dram_tensor("target", [E, D], BF16, kind="ExternalInput").ap()
    vt = nc.dram_tensor("vt", [D, NPAD], BF16, kind="ExternalInput").ap()
    out = nc.dram_tensor("out", [NG * G], F32, kind="ExternalOutput").ap()

    with tile.TileContext(nc) as tc, ExitStack() as ctx:
        tpool = ctx.enter_context(tc.tile_pool(name="tgt", bufs=1))
        vpool = ctx.enter_context(tc.tile_pool(name="v", bufs=1))
        spool = ctx.enter_context(tc.tile_pool(name="small", bufs=1))
        scr = ctx.enter_context(tc.tile_pool(name="scr", bufs=1))
        ps_w = ctx.enter_context(tc.tile_pool(name="psw", bufs=1, space="PSUM"))
        ps_sr = ctx.enter_context(tc.tile_pool(name="pssr", bufs=1, space="PSUM"))
        ps_c0 = ctx.enter_context(tc.tile_pool(name="psc0", bufs=1, space="PSUM"))
        ps_c1 = ctx.enter_context(tc.tile_pool(name="psc1", bufs=1, space="PSUM"))
        ps_da = ctx.enter_context(tc.tile_pool(name="psda", bufs=1, space="PSUM"))
        ps_db = ctx.enter_context(tc.tile_pool(name="psdb", bufs=1, space="PSUM"))
        ps_qa = ctx.enter_context(tc.tile_pool(name="psqa", bufs=1, space="PSUM"))
        ps_qb = ctx.enter_context(tc.tile_pool(name="psqb", bufs=1, space="PSUM"))

        tgt_sb = tpool.tile([128, ET, D], BF16, name="tgt_sb")
        tsq = scr.tile([128, ET, D], BF16, name="tsq")
        vt_sb = vpool.tile([128, A, NPAD], BF16, name="vt_sb")
        vsq = vpool.tile([128, A, NPAD], BF16, name="vsq")

        ssq_t = spool.tile([128, ET], F32, name="ssq_t")
        winv = spool.tile([128, ET], BF16, name="winv")
        s_colbf = spool.tile([128, A], BF16, name="s_colbf")
        eye2d = spool.tile([128, EYC], BF16, name="eye2d")
        dotw = spool.tile([128, A, EYC], BF16, name="dotw")
        warm_w = spool.tile([128, 1], BF16, name="warm_w")
        warm_x = spool.tile([128, G], BF16, name="warm_x")
        act_d = spool.tile([1, 1], F32, name="act_d")
        act_s = spool.tile([1, 1], F32, name="act_s")
        isv = [
            spool.tile([NP, G], F32, name="isva"),
            spool.tile([NP, G], F32, name="isvb"),
        ]
        res = [
            spool.tile([NP, G], F32, name="resa"),
            spool.tile([NP, G], F32, name="resb"),
        ]

        warm_ps = ps_w.tile([1, G], F32, name="warm_ps")
        scol_ps = [
            ps_c0.tile([128, 1], F32, name="scol0"),
            ps_c1.tile([128, 1], F32, name="scol1"),
        ]
        dot_ps = [
            ps_da.tile([NP, G], F32, name="dot_psa"),
            ps_db.tile([NP, G], F32, name="dot_psb"),
        ]
        sq_ps = [
            ps_qa.tile([NP, G], F32, name="sq_psa"),
            ps_qb.tile([NP, G], F32, name="sq_psb"),
        ]

        tgt_v = tgt.rearrange("(p j) d -> p j d", j=ET)
        vt_v = vt.rearrange("(a p) n -> p a n", p=128)
        out_v = out.rearrange("(g f) -> g f", f=G)

        def W(us):
            return tc.tile_wait_until(us / 1000.0)

        def bcols(b0, b1):
            return slice(b0 * G, b1 * G)

        ring_eng = {"S": nc.sync, "A": nc.scalar, "G": nc.gpsimd}

        # ---- DMA issues (high_priority: data-ready at t=0, always first)
        with tc.high_priority():
            for q in range(2):
                nc.sync.dma_start(
                    tgt_sb[:, q * H : (q + 1) * H, :],
                    tgt_v[:, q * H : (q + 1) * H, :],
                )
            for q in range(2, 4):
                nc.scalar.dma_start(
                    tgt_sb[:, q * H : (q + 1) * H, :],
                    tgt_v[:, q * H : (q + 1) * H, :],
                )
            for ring, bs, _sq, _t, early in CHUNKS:
                if not early:
                    continue
                sl = bcols(bs[0], bs[-1] + 1)
                ring_eng[ring].dma_start(vt_sb[:, :, sl], vt_v[:, :, sl])
        nc.vector.memset(act_d[:], 1.0)
        nc.scalar.activation(act_s[:], act_d[:], ARSQRT)  # pins the table load

        # ---- consts / on-device eye + dotw skeleton
        nc.vector.memset(warm_w[:], 1.0)
        nc.vector.memset(warm_x[:], 0.0)
        nc.vector.memset(eye2d[:], 0.0)
        nc.vector.memset(eye2d[:, 0 : EYC : NP + 1], 1.0)
        nc.vector.memset(dotw[:], 0.0)

        # ---- PE prewarm + fillers through the phase-A window
        for _ in range(WARM_MM):
            nc.tensor.matmul(warm_ps[:], warm_w[:], warm_x[:], start=True, stop=True)

        # ---- phase A ssq: q0/q1 DVE sq+red, q2 GP sq -> DVE red,
        # q3 split: 2 tiles ACT square+accum, 2 tiles GP sq -> DVE red
        for q in (0, 1):
            sl = slice(q * H, (q + 1) * H)
            nc.vector.tensor_mul(tsq[:, sl, :], tgt_sb[:, sl, :], tgt_sb[:, sl, :])
            with tc.high_priority():
                nc.vector.tensor_reduce(
                    ssq_t[:, sl], tsq[:, sl, :],
                    axis=mybir.AxisListType.X, op=mybir.AluOpType.add,
                )
                nc.scalar.activation(winv[:, sl], ssq_t[:, sl], ARSQRT)
        for q in (2, 3):
            sl = slice(q * H, (q + 1) * H)
            nc.gpsimd.tensor_mul(tsq[:, sl, :], tgt_sb[:, sl, :], tgt_sb[:, sl, :])
            with tc.high_priority():
                nc.vector.tensor_reduce(
                    ssq_t[:, sl], tsq[:, sl, :],
                    axis=mybir.AxisListType.X, op=mybir.AluOpType.add,
                )
                nc.scalar.activation(winv[:, sl], ssq_t[:, sl], ARSQRT)
        # GP bulk chunks issue after both gpsimd squares
        for ring, bs, _sq, _t, early in CHUNKS:
            if early or ring != "G":
                continue
            slc = bcols(bs[0], bs[-1] + 1)
            nc.gpsimd.dma_start(vt_sb[:, :, slc], vt_v[:, :, slc])

        # ---- phase A s-column matmuls (v3 scheme: 128-col weight loads
        # pipeline under the 1-col streams; output is s_col psum direct)
        # warm matmuls between quarters keep the HAM clock gate at 8/8
        # through the sparse phase-A region (stream-position placement,
        # NOT W-hints -- hinted fillers got woven into the node stream)
        for j in range(ET):
            for a in range(A):
                nc.tensor.matmul(
                    scol_ps[a][:],
                    tgt_sb[:, j, a * 128 : (a + 1) * 128],
                    winv[:, j : j + 1],
                    start=(j == 0),
                    stop=(j == ET - 1),
                )
            if j in (3, 7, 11):
                nc.tensor.matmul(
                    warm_ps[:], warm_w[:], warm_x[:], start=True, stop=True
                )
        for a in range(A):
            with tc.high_priority():
                nc.scalar.activation(
                    s_colbf[:, a : a + 1], scol_ps[a][:], COPY, scale=-1.0
                )
                # dotw diagonal = -s (stride NP+1 hits [r, r])
                nc.vector.tensor_copy(
                    dotw[:, a, 0 : EYC : NP + 1],
                    s_colbf[:, a : a + 1].broadcast_to([128, NP]),
                )

        # ---- node squares
        for ring, bs, sqe, t_arr, _e in CHUNKS:
            if sqe == "S":
                for b in bs:
                    with W(t_arr + 0.2):
                        nc.scalar.activation(
                            vsq[:, :, b * G : (b + 1) * G],
                            vt_sb[:, :, bcols(b, b + 1)],
                            SQUARE,
                        )
            else:
                with W(t_arr + 0.2):
                    nc.vector.tensor_mul(
                        vsq[:, :, bs[0] * G : (bs[-1] + 1) * G],
                        vt_sb[:, :, bcols(bs[0], bs[-1] + 1)],
                        vt_sb[:, :, bcols(bs[0], bs[-1] + 1)],
                    )

        # ---- PE node matmuls
        def pair_of(b):
            return (0, b) if b < NP else (1, b - NP)

        def emit_mms(order, ps, lhs_for, t_of):
            first_seen = {0: True, 1: True}
            remaining = {0: sum(1 for b in order if b < NP),
                         1: sum(1 for b in order if b >= NP)}
            for b in order:
                p, r = pair_of(b)
                remaining[p] -= 1
                with W(t_of(b)):
                    for a in range(A):
                        nc.tensor.matmul(
                            ps[p][:],
                            lhs_for(a, r),
                            (vsq[:, a, b * G : (b + 1) * G]
                             if ps is sq_ps
                             else vt_sb[:, a, bcols(b, b + 1)]),
                            start=(first_seen[p] and a == 0),
                            stop=(remaining[p] == 0 and a == 1),
                        )
                first_seen[p] = False

        def emit_split(pre, dots, post):
            ssq_all = pre + post
            fs_d = {0: True, 1: True}
            fs_q = {0: True, 1: True}
            rem_d = {0: sum(1 for b in dots if b < NP),
                     1: sum(1 for b in dots if b >= NP)}
            rem_q = {0: sum(1 for b in ssq_all if b < NP),
                     1: sum(1 for b in ssq_all if b >= NP)}

            def one(b, ps, fs, rem, lhs_for, t):
                p, r = pair_of(b)
                rem[p] -= 1
                with W(t):
                    for a in range(A):
                        nc.tensor.matmul(
                            ps[p][:],
                            lhs_for(a, r),
                            (vsq[:, a, b * G : (b + 1) * G]
                             if ps is sq_ps
                             else vt_sb[:, a, bcols(b, b + 1)]),
                            start=(fs[p] and a == 0),
                            stop=(rem[p] == 0 and a == 1),
                        )
                fs[p] = False

            dl = lambda a, r: dotw[:, a, r * NP : (r + 1) * NP]
            ql = lambda a, r: eye2d[:, r * NP : (r + 1) * NP]
            for b in pre:
                one(b, sq_ps, fs_q, rem_q, ql, ARRIVE[b] + 0.7)
            for b in dots:
                one(b, dot_ps, fs_d, rem_d, dl, max(S_READY, ARRIVE[b] + 0.2))
            for b in post:
                one(b, sq_ps, fs_q, rem_q, ql, max(S_READY, ARRIVE[b] + 0.7))

        for _ in range(2):
            nc.tensor.matmul(warm_ps[:], warm_w[:], warm_x[:], start=True, stop=True)
        emit_split([0, 1, 2], DOT_ORDER, [3, 4, 9, 10, 11, 6, 7, 8, 5, 13, 12])
        # tails
        for p, t_tail in ((0, 23.4), (1, 23.9)):
            with W(t_tail):
                nc.scalar.activation(isv[p][:], sq_ps[p][:], ARSQRT)
                nc.vector.tensor_mul(res[p][:], dot_ps[p][:], isv[p][:])
                nc.sync.dma_start(out_v[p * NP : (p + 1) * NP, :], res[p][:])

    nc.compile()
    return nc


def _get_nc():
    if "nc" not in _cache:
        _cache["nc"] = _build()
    return _cache["nc"]


def _host_inputs(target, node_emb):
    tgt_bf = np.ascontiguousarray(np.asarray(target, dtype=np.float32)).astype(BF)
    node_emb = np.asarray(node_emb, dtype=np.float32)

    in_maps = []
    for c in range(N_CORES):
        shard = np.empty((NPAD, D), dtype=np.float32)
        shard[:NPC] = node_emb[c * NPC : (c + 1) * NPC]
        shard[NPC:] = node_emb[: NPAD - NPC]  # pad with real rows (no 0-norm)
        vtp = np.ascontiguousarray(shard.T.astype(BF))
        in_maps.append({"target": tgt_bf, "vt": vtp})
    return in_maps


def run(pred, target, node_emb, trace=False, **trace_kwargs):
    """Returns (full_output [50000] f32, BassKernelResults)."""
    nc = _get_nc()
    in_maps = _host_inputs(target, node_emb)
    res = bass_utils.run_bass_kernel_spmd(
        nc, in_maps, list(range(N_CORES)), trace=trace, **trace_kwargs
    )
    parts = [res.results[c]["out"][:NPC] for c in range(N_CORES)]
    return np.concatenate(parts).astype(np.float32), res


def kernel(pred, target, node_emb):
    out, _ = run(pred, target, node_emb)
    return out


# revision 32
# speedup vs baseline: 1.0117x; 1.0117x over previous
"""ContrastiveDist kernel for TRN2 (8 NeuronCores, SPMD) -- v4.9.

Measured: 38732 ns HW exec (core 0), rel err 3.23e-3 (vs 44673 ns
baseline v3).

out[n] = sum_e -(t_e . v_n) / (||t_e|| * ||v_n|| + eps)
       = (s . v_n) / ||v_n||      with s = -sum_e t_e / ||t_e||
(eps shifts the result by ~4e-11 relative -- dropped.)

Schedule design (from the v3 / v4.0-v4.4 traces):
 * THREE DMA queues: SP HWDGE (nc.sync), ACT HWDGE (nc.scalar), GPSIMD
   SWDGE (nc.gpsimd); ~285 GB/s aggregate HBM-bound, round-robin per
   packet.  tgt quarters head both HWDGE rings.  ALL DMA issues carry
   high_priority: they are data-ready at sim t=0, so the static
   scheduler always places them ahead of (mispredicted) compute --
   v4.4's XD issue sat behind phase-A ACT work until 19.6us.
 * eye and dotw are built ON DEVICE with strided memsets/copies
   (diagonal of a [128, 49] tile = stride-8 free-axis slice) -- no eye
   DMA, no eye dependency in the s chain, and the GP ring's first chunk
   is a clean single node block.
 * target entity-major [128e, 16, 256d], 4 quarters: DVE square+reduce
   for q0/q1, GPSIMD square -> DVE reduce for q2/q3 (ACT Square with
   accum_out works but costs 769ns/tile and jams the ACT queue ahead
   of DMA issues -- removed in v4.7).  ACT Abs_reciprocal_sqrt emits
   winv in BF16 directly; the s sign folds into ACT Copy(scale=-1)
   column copies.
 * GP ring: first block immediately, bulk chunks issued after the
   gpsimd phase-A square so tgt keeps the early bandwidth.
 * PE prewarm matmuls run while the first DMAs stream (HAM clock gate
   wants ~3.4us of activity).  NOTE: W-hinted "filler" matmuls placed
   mid-kernel got mis-scheduled into the node matmul stream and COST
   ~1-2us (v4.7) -- do not re-add.
 * phase-A s accumulation uses the v3 column scheme: 32 matmuls with
   [128e, 128d] weight loads and 1-column rhs pipeline at ~27ns each
   and write s_col psum directly (no row->column conversion chain).
 * psum pairs: blocks 0-6 / 7-13; block-diag lhsT routes block b to its
   psum row; tails are ACT arsqrt [7,448] + one DVE mul; outs on SP.
 * fused DVE tensor_tensor_reduce is NOT used anywhere (locks up HW).
"""

import numpy as np
import ml_dtypes
from contextlib import ExitStack

import concourse.bacc as bacc
import concourse.bass as bass
import concourse.mybir as mybir
import concourse.tile as tile
from concourse import bass_utils

E, D = 2048, 256
N_FULL = 50000
N_CORES = 8
NPC = N_FULL // N_CORES
G = 448
NG = 14
NPAD = G * NG
NP = 7
A = 2
ET = E // 128
EYC = NP * NP            # eye tile columns (on-device)
TQ = 4
H = ET // TQ
WARM_MM = 6

# node chunks: (ring, [blocks], square engine 'V'/'S', est arrival us,
#               issue early?)
CHUNKS = [
    ("G", [0], "V", 12.1, True),            # GE
    ("G", [1, 2], "S", 14.2, True),         # GA
    ("G", [3, 4], "V", 20.0, False),        # GB (issued after gp TT q3)
    ("G", [5], "V", 22.2, False),           # GC
    ("S", [6, 7, 8], "V", 20.0, True),      # SA
    ("A", [9, 10, 11], "S", 20.0, True),    # XA
    ("S", [12], "S", 23.0, True),           # SD
    ("A", [13], "V", 23.0, True),           # XD
]
DOT_ORDER = [0, 1, 2, 3, 4, 6, 7, 8, 9, 10, 11, 5, 12, 13]
SSQ_ORDER = [0, 1, 2, 3, 4, 6, 7, 8, 9, 10, 11, 5, 12, 13]
ARRIVE = {}
for _ring, _bs, _sq, _t, _e in CHUNKS:
    for _b in _bs:
        ARRIVE[_b] = _t
S_READY = 18.4

F32 = mybir.dt.float32
BF16 = mybir.dt.bfloat16
BF = ml_dtypes.bfloat16
ARSQRT = mybir.ActivationFunctionType.Abs_reciprocal_sqrt
SQUARE = mybir.ActivationFunctionType.Square
COPY = mybir.ActivationFunctionType.Copy

_cache = {}


def _build():
    nc = bacc.Bacc(
        "TRN2",
        target_bir_lowering=False,
        debug=False,
        enable_asserts=True,
        num_devices=N_CORES,
    )
    tgt = nc.dram_tensor("target", [E, D], BF16, kind="ExternalInput").ap()
    vt = nc.dram_tensor("vt", [D, NPAD], BF16, kind="ExternalInput").ap()
    out = nc.dram_tensor("out", [NG * G], F32, kind="ExternalOutput").ap()

    with tile.TileContext(nc) as tc, ExitStack() as ctx:
        tpool = ctx.enter_context(tc.tile_pool(name="tgt", bufs=1))
        vpool = ctx.enter_context(tc.tile_pool(name="v", bufs=1))
        spool = ctx.enter_context(tc.tile_pool(name="small", bufs=1))
        scr = ctx.enter_context(tc.tile_pool(name="scr", bufs=1))
        ps_w = ctx.enter_context(tc.tile_pool(name="psw", bufs=1, space="PSUM"))
        ps_sr = ctx.enter_context(tc.tile_pool(name="pssr", bufs=1, space="PSUM"))
        ps_c0 = ctx.enter_context(tc.tile_pool(name="psc0", bufs=1, space="PSUM"))
        ps_c1 = ctx.enter_context(tc.tile_pool(name="psc1", bufs=1, space="PSUM"))
        ps_da = ctx.enter_context(tc.tile_pool(name="psda", bufs=1, space="PSUM"))
        ps_db = ctx.enter_context(tc.tile_pool(name="psdb", bufs=1, space="PSUM"))
        ps_qa = ctx.enter_context(tc.tile_pool(name="psqa", bufs=1, space="PSUM"))
        ps_qb = ctx.enter_context(tc.tile_pool(name="psqb", bufs=1, space="PSUM"))

        tgt_sb = tpool.tile([128, ET, D], BF16, name="tgt_sb")
        tsq = scr.tile([128, ET, D], BF16, name="tsq")
        vt_sb = vpool.tile([128, A, NPAD], BF16, name="vt_sb")
        vsq = vpool.tile([128, A, NPAD], BF16, name="vsq")

        ssq_t = spool.tile([128, ET], F32, name="ssq_t")
        winv = spool.tile([128, ET], BF16, name="winv")
        s_colbf = spool.tile([128, A], BF16, name="s_colbf")
        eye2d = spool.tile([128, EYC], BF16, name="eye2d")
        dotw = spool.tile([128, A, EYC], BF16, name="dotw")
        warm_w = spool.tile([128, 1], BF16, name="warm_w")
        warm_x = spool.tile([128, G], BF16, name="warm_x")
        act_d = spool.tile([1, 1], F32, name="act_d")
        act_s = spool.tile([1, 1], F32, name="act_s")
        isv = [
            spool.tile([NP, G], F32, name="isva"),
            spool.tile([NP, G], F32, name="isvb"),
        ]
        res = [
            spool.tile([NP, G], F32, name="resa"),
            spool.tile([NP, G], F32, name="resb"),
        ]

        warm_ps = ps_w.tile([1, G], F32, name="warm_ps")
        scol_ps = [
            ps_c0.tile([128, 1], F32, name="scol0"),
            ps_c1.tile([128, 1], F32, name="scol1"),
        ]
        dot_ps = [
            ps_da.tile([NP, G], F32, name="dot_psa"),
            ps_db.tile([NP, G], F32, name="dot_psb"),
        ]
        sq_ps = [
            ps_qa.tile([NP, G], F32, name="sq_psa"),
            ps_qb.tile([NP, G], F32, name="sq_psb"),
        ]

        tgt_v = tgt.rearrange("(p j) d -> p j d", j=ET)
        vt_v = vt.rearrange("(a p) n -> p a n", p=128)
        out_v = out.rearrange("(g f) -> g f", f=G)

        def W(us):
            return tc.tile_wait_until(us / 1000.0)

        def bcols(b0, b1):
            return slice(b0 * G, b1 * G)

        ring_eng = {"S": nc.sync, "A": nc.scalar, "G": nc.gpsimd}

        # ---- DMA issues (high_priority: data-ready at t=0, always first)
        with tc.high_priority():
            for q in range(2):
                nc.sync.dma_start(
                    tgt_sb[:, q * H : (q + 1) * H, :],
                    tgt_v[:, q * H : (q + 1) * H, :],
                )
            for q in range(2, 4):
                nc.scalar.dma_start(
                    tgt_sb[:, q * H : (q + 1) * H, :],
                    tgt_v[:, q * H : (q + 1) * H, :],
                )
            for ring, bs, _sq, _t, early in CHUNKS:
                if not early:
                    continue
                sl = bcols(bs[0], bs[-1] + 1)
                ring_eng[ring].dma_start(vt_sb[:, :, sl], vt_v[:, :, sl])
        nc.vector.memset(act_d[:], 1.0)
        nc.scalar.activation(act_s[:], act_d[:], ARSQRT)  # pins the table load

        # ---- consts / on-device eye + dotw skeleton
        nc.vector.memset(warm_w[:], 1.0)
        nc.vector.memset(warm_x[:], 0.0)
        nc.vector.memset(eye2d[:], 0.0)
        nc.vector.memset(eye2d[:, 0 : EYC : NP + 1], 1.0)
        nc.vector.memset(dotw[:], 0.0)

        # ---- PE prewarm + fillers through the phase-A window
        for _ in range(WARM_MM):
            nc.tensor.matmul(warm_ps[:], warm_w[:], warm_x[:], start=True, stop=True)

        # ---- phase A ssq: q0/q1 DVE sq+red, q2 GP sq -> DVE red,
        # q3 split: 2 tiles ACT square+accum, 2 tiles GP sq -> DVE red
        for q in (0, 1):
            sl = slice(q * H, (q + 1) * H)
            nc.vector.tensor_mul(tsq[:, sl, :], tgt_sb[:, sl, :], tgt_sb[:, sl, :])
            with tc.high_priority():
                nc.vector.tensor_reduce(
                    ssq_t[:, sl], tsq[:, sl, :],
                    axis=mybir.AxisListType.X, op=mybir.AluOpType.add,
                )
                nc.scalar.activation(winv[:, sl], ssq_t[:, sl], ARSQRT)
        sl = slice(2 * H, 3 * H)
        nc.gpsimd.tensor_mul(tsq[:, sl, :], tgt_sb[:, sl, :], tgt_sb[:, sl, :])
        with tc.high_priority():
            nc.vector.tensor_reduce(
                ssq_t[:, sl], tsq[:, sl, :],
                axis=mybir.AxisListType.X, op=mybir.AluOpType.add,
            )
            nc.scalar.activation(winv[:, sl], ssq_t[:, sl], ARSQRT)
        sl = slice(3 * H, ET)
        nc.gpsimd.tensor_mul(tsq[:, sl, :], tgt_sb[:, sl, :], tgt_sb[:, sl, :])
        for j0 in (3 * H, 3 * H + 2):
            slh = slice(j0, j0 + 2)
            with tc.high_priority():
                nc.vector.tensor_reduce(
                    ssq_t[:, slh], tsq[:, slh, :],
                    axis=mybir.AxisListType.X, op=mybir.AluOpType.add,
                )
                nc.scalar.activation(winv[:, slh], ssq_t[:, slh], ARSQRT)
        # GP bulk chunks issue after both gpsimd squares
        for ring, bs, _sq, _t, early in CHUNKS:
            if early or ring != "G":
                continue
            slc = bcols(bs[0], bs[-1] + 1)
            nc.gpsimd.dma_start(vt_sb[:, :, slc], vt_v[:, :, slc])

        # ---- phase A s-column matmuls (v3 scheme: 128-col weight loads
        # pipeline under the 1-col streams; output is s_col psum direct)
        for j in range(ET):
            for a in range(A):
                nc.tensor.matmul(
                    scol_ps[a][:],
                    tgt_sb[:, j, a * 128 : (a + 1) * 128],
                    winv[:, j : j + 1],
                    start=(j == 0),
                    stop=(j == ET - 1),
                )
            if j in (3, 7, 9, 11, 13):
                # thin 64-col fillers (~120ns cold) keep the HAM busy
                # through the sparse phase-A region without the fat
                # 448-col slot cost that regressed v5.1
                for _ in range(3):
                    nc.tensor.matmul(
                        warm_ps[:, 0:64], warm_w[:], warm_x[:, 0:64],
                        start=True, stop=True,
                    )
        for a in range(A):
            with tc.high_priority():
                nc.scalar.activation(
                    s_colbf[:, a : a + 1], scol_ps[a][:], COPY, scale=-1.0
                )
                # dotw diagonal = -s (stride NP+1 hits [r, r])
                nc.vector.tensor_copy(
                    dotw[:, a, 0 : EYC : NP + 1],
                    s_colbf[:, a : a + 1].broadcast_to([128, NP]),
                )

        # ---- node squares
        for ring, bs, sqe, t_arr, _e in CHUNKS:
            if sqe == "S":
                for b in bs:
                    with W(t_arr + 0.2):
                        nc.scalar.activation(
                            vsq[:, :, b * G : (b + 1) * G],
                            vt_sb[:, :, bcols(b, b + 1)],
                            SQUARE,
                        )
            else:
                with W(t_arr + 0.2):
                    nc.vector.tensor_mul(
                        vsq[:, :, bs[0] * G : (bs[-1] + 1) * G],
                        vt_sb[:, :, bcols(bs[0], bs[-1] + 1)],
                        vt_sb[:, :, bcols(bs[0], bs[-1] + 1)],
                    )

        # ---- PE node matmuls
        def pair_of(b):
            return (0, b) if b < NP else (1, b - NP)

        def emit_mms(order, ps, lhs_for, t_of):
            first_seen = {0: True, 1: True}
            remaining = {0: sum(1 for b in order if b < NP),
                         1: sum(1 for b in order if b >= NP)}
            for b in order:
                p, r = pair_of(b)
                remaining[p] -= 1
                with W(t_of(b)):
                    for a in range(A):
                        nc.tensor.matmul(
                            ps[p][:],
                            lhs_for(a, r),
                            (vsq[:, a, b * G : (b + 1) * G]
                             if ps is sq_ps
                             else vt_sb[:, a, bcols(b, b + 1)]),
                            start=(first_seen[p] and a == 0),
                            stop=(remaining[p] == 0 and a == 1),
                        )
                first_seen[p] = False

        def emit_split(pre, dots, post):
            ssq_all = pre + post
            fs_d = {0: True, 1: True}
            fs_q = {0: True, 1: True}
            rem_d = {0: sum(1 for b in dots if b < NP),
                     1: sum(1 for b in dots if b >= NP)}
            rem_q = {0: sum(1 for b in ssq_all if b < NP),
                     1: sum(1 for b in ssq_all if b >= NP)}

            def one(b, ps, fs, rem, lhs_for, t):
                p, r = pair_of(b)
                rem[p] -= 1
                with W(t):
                    for a in range(A):
                        nc.tensor.matmul(
                            ps[p][:],
                            lhs_for(a, r),
                            (vsq[:, a, b * G : (b + 1) * G]
                             if ps is sq_ps
                             else vt_sb[:, a, bcols(b, b + 1)]),
                            start=(fs[p] and a == 0),
                            stop=(rem[p] == 0 and a == 1),
                        )
                fs[p] = False

            dl = lambda a, r: dotw[:, a, r * NP : (r + 1) * NP]
            ql = lambda a, r: eye2d[:, r * NP : (r + 1) * NP]
            for b in pre:
                one(b, sq_ps, fs_q, rem_q, ql, ARRIVE[b] + 0.7)
            for b in dots:
                one(b, dot_ps, fs_d, rem_d, dl, max(S_READY, ARRIVE[b] + 0.2))
            for b in post:
                one(b, sq_ps, fs_q, rem_q, ql, max(S_READY, ARRIVE[b] + 0.7))

        emit_split([0, 1, 2], DOT_ORDER, [3, 4, 9, 10, 11, 6, 7, 8, 5, 13, 12])
        # tails
        for p, t_tail in ((0, 23.4), (1, 23.9)):
            with W(t_tail):
                nc.scalar.activation(isv[p][:], sq_ps[p][:], ARSQRT)
                nc.vector.tensor_mul(res[p][:], dot_ps[p][:], isv[p][:])
                nc.sync.dma_start(out_v[p * NP : (p + 1) * NP, :], res[p][:])

    nc.compile()
    return nc


def _get_nc():
    if "nc" not in _cache:
        _cache["nc"] = _build()
    return _cache["nc"]


def _host_inputs(target, node_emb):
    tgt_bf = np.ascontiguousarray(np.asarray(target, dtype=np.float32)).astype(BF)
    node_emb = np.asarray(node_emb, dtype=np.float32)

    in_maps = []
    for c in range(N_CORES):
        shard = np.empty((NPAD, D), dtype=np.float32)
        shard[:NPC] = node_emb[c * NPC : (c + 1) * NPC]
        shard[NPC:] = node_emb[: NPAD - NPC]  # pad with real rows (no 0-norm)
        vtp = np.ascontiguousarray(shard.T.astype(BF))
        in_maps.append({"target": tgt_bf, "vt": vtp})
    return in_maps


def run(pred, target, node_emb, trace=False, **trace_kwargs):
    """Returns (full_output [50000] f32, BassKernelResults)."""
    nc = _get_nc()
    in_maps = _host_inputs(target, node_emb)
    res = bass_utils.run_bass_kernel_spmd(
        nc, in_maps, list(range(N_CORES)), trace=trace, **trace_kwargs
    )
    parts = [res.results[c]["out"][:NPC] for c in range(N_CORES)]
    return np.concatenate(parts).astype(np.float32), res


def kernel(pred, target, node_emb):
    out, _ = run(pred, target, node_emb)
    return out
